# revision 1
# baseline (speedup 1.0000x reference)
"""Trainium2 Bass kernel for nn_Augmenter (color jitter + translate + cutout).

Contract: kernel(**inputs) takes FULL unsharded numpy inputs
(imgs [128,3,256,256] f32, br/sat/con [128,1,1,1] f32,
 tx/ty/cx/cy [128,1,1] i32) and returns the FULL output [128,3,256,256] f32.

Internally: shard batch over 8 NeuronCores (16 images each), run one SPMD
Bass/Tile kernel via run_bass_kernel_spmd, reassemble on host.

Math (per image, derived from the reference):
  b = br-0.5, s = 2*sat, c = con+0.5
  color:  x3 = A*x + Bp*MC + D
          A  = c*s
          Bp = c*(1-s)/3          (MC = sum over the 3 channels of x)
          D  = (1-c)*m0 + b       (m0 = mean over all pixels+channels of x)
  translate by (txs,tys) = (tx-32, ty-32) with zero fill
  cutout: zero rows [max(0,cx-64), min(255,cx+63)] x cols [..cy..]

Implementation notes:
  * The translation (rows AND cols) is done by ONE flat dynamic-offset DMA
    store per plane: writing the color-transformed plane at flat offset
    -(txs*256+tys) relative to a fixed extraction window inside a padded
    output slot. Column wrap-around garbage is zeroed on-chip by a
    column-validity vector folded into the mask; uncovered head/tail rows
    rely on the harness pre-zeroing ExternalOutput buffers (the native
    run_bass_kernel_spmd path documents this; bass2jax donates zero buffers).
  * mask'[r,s] = rc[r]*ccs[s] - cvs[s]  (= -mask) is built on the otherwise
    idle TensorEngine as accumulated rank-1 matmuls into PSUM.
    The sign is folded into negated A/Bp/D so out = (-x3)*mask' = x3*mask.
  * SBUF plane layout: [128 partitions, 512 free]; partition p holds image
    rows 2p and 2p+1 (flat row-major <-> (p, free) is the identity), so both
    load and store DMAs are fully contiguous (2KB per partition).
"""

import numpy as np

import concourse.bacc as bacc
import concourse.bass as bass
import concourse.mybir as mybir
import concourse.tile as tile
from concourse.bass_isa import ReduceOp
from concourse.bass_utils import run_bass_kernel_spmd

F32 = mybir.dt.float32
I32 = mybir.dt.int32
OP = mybir.AluOpType
AF = mybir.ActivationFunctionType

N_CORES = 8
B_FULL = 128
IMGS_PER_CORE = B_FULL // N_CORES  # 16
C, H, W = 3, 256, 256
PLANE = H * W  # 65536

# Padded output slot geometry. Dynamic store offset within a slot is
# off = MARG - s0, s0 = txs*256 + tys in [-8224, 8224], MARG = 8448.
# off in [224, 16672]; the write occupies [off, off+PLANE) of the slot.
MARG = 8448
SLOT = PLANE + MARG  # 73984 stride; margins shared between neighbours
OFF_MIN, OFF_MAX = 224, 16672


def _build_kernel(n_imgs: int, repeat: int = 1):
    """Build + compile the per-core SPMD program.

    repeat > 1 re-emits the per-image pipeline (identical work+writes) for
    amortized wall-clock timing; output is unchanged.
    """
    nc = bacc.Bacc(
        "TRN2",
        target_bir_lowering=False,
        debug=False,
        enable_asserts=False,
        num_devices=N_CORES,
    )
    n_planes = n_imgs * C
    out_flat = (n_planes - 1) * SLOT + OFF_MAX + PLANE

    imgs_t = nc.dram_tensor("imgs", [n_planes, PLANE], F32, kind="ExternalInput")
    # params twice: row layout [1, 8*n] and column layout [n, 8]
    prmr_t = nc.dram_tensor("prmr", [1, 8 * n_imgs], F32, kind="ExternalInput")
    prmc_t = nc.dram_tensor("prmc", [n_imgs, 8], F32, kind="ExternalInput")
    out_t = nc.dram_tensor("out", [out_flat], F32, kind="ExternalOutput")
    imgs = imgs_t.ap()
    prmr = prmr_t.ap()
    prmc = prmc_t.ap()
    out = out_t.ap()

    with tile.TileContext(nc) as tc:
        with (
            tc.tile_pool(name="const", bufs=1) as cpool,
            tc.tile_pool(name="xin", bufs=9) as xpool,
            tc.tile_pool(name="tsum", bufs=2) as tpool,
            tc.tile_pool(name="mc", bufs=3) as mcpool,
            tc.tile_pool(name="tmp", bufs=3) as tmppool,
            tc.tile_pool(name="msk", bufs=3) as mskpool,
            tc.tile_pool(name="yy", bufs=3) as ypool,
            tc.tile_pool(name="oo", bufs=4) as opool,
            tc.tile_pool(name="sm", bufs=8) as smpool,
            tc.tile_pool(name="vr", bufs=6) as vrpool,
            tc.tile_pool(name="ps", bufs=2, space="PSUM") as pspool,
        ):
            V = nc.vector

            # ---------------- one-time setup ----------------
            io_i = cpool.tile([n_imgs, 256], I32)
            nc.gpsimd.iota(io_i, pattern=[[1, 256]], base=0, channel_multiplier=0)
            IO = cpool.tile([n_imgs, 256], F32)
            V.tensor_copy(IO, io_i)

            ONES = cpool.tile([1, 128], F32)
            V.memset(ONES, 1.0)

            # static scatter-offset skeleton: 512*p + SLOT*c  (c = channel)
            # (iota steps are int16-limited, so compose from two small iotas)
            ic3_i = cpool.tile([128, 3], I32)
            nc.gpsimd.iota(ic3_i, pattern=[[1, 3]], base=0, channel_multiplier=0)
            ip_i = cpool.tile([128, 1], I32)
            nc.gpsimd.iota(ip_i, pattern=[[1, 1]], base=0, channel_multiplier=512)
            IC3f = cpool.tile([128, 3], F32)
            V.tensor_copy(IC3f, ic3_i)
            IPf = cpool.tile([128, 1], F32)
            V.tensor_copy(IPf, ip_i)
            ICSf = cpool.tile([128, 3], F32)
            V.tensor_scalar(ICSf, IC3f, float(SLOT), IPf[:, 0:1], OP.mult, OP.add)

            # row-layout params [1, 8*n]: slot g*n_imgs + i = param g of image i
            Pr = cpool.tile([1, 8 * n_imgs], F32)
            nc.scalar.dma_start(Pr, prmr)
            n = n_imgs
            BRr, CONr = Pr[:, 0 * n : 1 * n], Pr[:, 2 * n : 3 * n]
            SATr = Pr[:, 1 * n : 2 * n]
            TXr, TYr = Pr[:, 3 * n : 4 * n], Pr[:, 4 * n : 5 * n]

            # column-layout params [n, 8]
            Pc = cpool.tile([n_imgs, 8], F32)
            nc.scalar.dma_start(Pc, prmc)
            TXc, TYc = Pc[:, 3:4], Pc[:, 4:5]
            CXc, CYc = Pc[:, 5:6], Pc[:, 6:7]

            # --- row-layout crunch: negA/negBp/offbase (-> P3), ep, bpp ---
            # P3 row: [1, 4*n]; image i slots [4i,4i+4) = negA, negBp, negD, offbase
            P3 = cpool.tile([1, 4 * n_imgs], F32)
            negA = P3[:, 0 : 4 * n : 4]
            negBp = P3[:, 1 : 4 * n : 4]
            offb = P3[:, 3 : 4 * n : 4]
            ROW = cpool.tile([1, 4 * n_imgs], F32)
            cf = ROW[:, 0 * n : 1 * n]
            ep = ROW[:, 1 * n : 2 * n]
            bpp = ROW[:, 2 * n : 3 * n]
            rt = ROW[:, 3 * n : 4 * n]

            V.tensor_scalar(cf, CONr, 1.0, 0.5, OP.mult, OP.add)
            V.tensor_scalar(ep, cf, 1.0 / 196608.0, -1.0 / 196608.0, OP.mult, OP.add)
            V.tensor_scalar(bpp, BRr, -1.0, 0.5, OP.mult, OP.add)
            V.tensor_scalar(rt, SATr, 2.0, None, OP.mult)
            V.tensor_tensor(rt, cf, rt, OP.mult)  # A = c*2sat
            V.tensor_scalar(negA, rt, -1.0, None, OP.mult)
            V.tensor_tensor(rt, rt, cf, OP.subtract)  # A - c
            V.tensor_scalar(negBp, rt, 1.0 / 3.0, None, OP.mult)

            # scatter offset base: MARG - s0 = 16672 - 256*tx - ty
            V.tensor_scalar(offb, TXr, -256.0, 16672.0, OP.mult, OP.add)
            V.tensor_tensor(offb, offb, TYr, OP.subtract)

            # --- column-layout crunch + batched mask vectors [n, 256] ---
            COL = cpool.tile([n_imgs, 6], F32)
            txs_c = COL[:, 0:1]
            tys_c = COL[:, 1:2]
            lo = COL[:, 2:3]
            hi = COL[:, 3:4]
            V.tensor_scalar(txs_c, TXc, 32.0, None, OP.subtract)
            V.tensor_scalar(tys_c, TYc, 32.0, None, OP.subtract)

            RC = cpool.tile([n_imgs, 256], F32)   # row in (shifted) cut range
            CCS = cpool.tile([n_imgs, 256], F32)  # col in (shifted) cut range
            NCV = cpool.tile([n_imgs, 256], F32)  # -(col valid)
            e1 = cpool.tile([n_imgs, 256], F32)

            # rows: lo_x = max(0,cx-64)+txs ; hi_x = min(255,cx+63)+txs
            V.tensor_scalar(lo, CXc, 64.0, 0.0, OP.subtract, OP.max)
            V.tensor_tensor(lo, lo, txs_c, OP.add)
            V.tensor_scalar(hi, CXc, 63.0, 255.0, OP.add, OP.min)
            V.tensor_tensor(hi, hi, txs_c, OP.add)
            V.tensor_scalar(e1, IO, hi, None, OP.is_le)
            V.scalar_tensor_tensor(RC, IO, lo, e1, OP.is_ge, OP.logical_and)

            # cols: lo_y = max(0,cy-64)+tys ; hi_y = min(255,cy+63)+tys
            V.tensor_scalar(lo, CYc, 64.0, 0.0, OP.subtract, OP.max)
            V.tensor_tensor(lo, lo, tys_c, OP.add)
            V.tensor_scalar(hi, CYc, 63.0, 255.0, OP.add, OP.min)
            V.tensor_tensor(hi, hi, tys_c, OP.add)
            V.tensor_scalar(e1, IO, hi, None, OP.is_le)
            V.scalar_tensor_tensor(CCS, IO, lo, e1, OP.is_ge, OP.logical_and)

            # -(tys <= s < tys+256)
            V.tensor_scalar(hi, tys_c, 256.0, None, OP.add)
            V.tensor_scalar(e1, IO, hi, None, OP.is_lt)
            V.scalar_tensor_tensor(NCV, IO, tys_c, e1, OP.is_ge, OP.logical_and)
            V.tensor_scalar(NCV, NCV, -1.0, None, OP.mult)

            # ---------------- per-image pipeline ----------------
            for rep in range(repeat):
              for i in range(n_imgs):
                  x = [
                      xpool.tile([128, 512], F32, tag="x", name=f"x{i}_{c}")
                      for c in range(C)
                  ]
                  for c in range(C):
                      nc.scalar.dma_start(
                          x[c], imgs[i * C + c].rearrange("(p f) -> p f", p=128)
                      )

                  t = tpool.tile([128, 512], F32, tag="t")
                  V.tensor_tensor(t, x[0], x[1], OP.add)
                  MC = mcpool.tile([128, 512], F32, tag="mc")
                  mcp = smpool.tile([128, 1], F32, tag="mcp")
                  V.scalar_tensor_tensor(MC, t, 1.0, x[2], OP.mult, OP.add, accum_out=mcp)
                  m0r = smpool.tile([128, 1], F32, tag="m0r")
                  nc.gpsimd.partition_all_reduce(m0r, mcp, 128, ReduceOp.add)
                  # negD = ep*SUM + bpp  -> P3[0, 4i+2]
                  V.scalar_tensor_tensor(
                      P3[:, 4 * i + 2 : 4 * i + 3],
                      m0r[0:1, 0:1],
                      ep[:, i : i + 1],
                      bpp[:, i : i + 1],
                      OP.mult,
                      OP.add,
                  )
                  Sb = smpool.tile([128, 4], F32, tag="sb")
                  nc.gpsimd.partition_broadcast(Sb, P3[:, 4 * i : 4 * i + 4])

                  # scatter offsets: 512*p + SLOT*c + offbase + 3*i*SLOT
                  offtf = smpool.tile([128, 3], F32, tag="offtf")
                  V.tensor_scalar(
                      offtf, ICSf, Sb[:, 3:4], float(3 * i * SLOT), OP.add, OP.add
                  )
                  offt = smpool.tile([128, 3], I32, tag="offt")
                  V.tensor_copy(offt, offtf)

                  # tmp' = negBp*MC + negD   (ScalarE)
                  tmp = tmppool.tile([128, 512], F32, tag="tmp")
                  nc.scalar.activation(
                      tmp, MC, AF.Identity, bias=Sb[:, 2:3], scale=Sb[:, 1:2]
                  )

                  # stage this image's mask vectors at partition 0 (tiny DMAs)
                  rcr = vrpool.tile([1, 256], F32, tag="rcr")
                  ccr = vrpool.tile([1, 256], F32, tag="ccr")
                  nvr = vrpool.tile([1, 256], F32, tag="nvr")
                  nc.sync.dma_start(rcr, RC[i : i + 1, :])
                  nc.sync.dma_start(ccr, CCS[i : i + 1, :])
                  nc.sync.dma_start(nvr, NCV[i : i + 1, :])

                  # mask' = rc x ccs - 1 x cvs   (PE, rank-2 into PSUM)
                  pm = pspool.tile([128, 512], F32, tag="pm")
                  for b in range(2):
                      half = pm[:, b * 256 : (b + 1) * 256]
                      nc.tensor.matmul(
                          half,
                          lhsT=rcr[:, b : 256 : 2],  # rc[2p+b] over p
                          rhs=ccr,
                          start=True,
                          stop=False,
                      )
                      nc.tensor.matmul(half, lhsT=ONES, rhs=nvr, start=False, stop=True)
                  msk = mskpool.tile([128, 512], F32, tag="msk")
                  nc.scalar.activation(msk, pm, AF.Copy)  # PSUM -> SBUF

                  for c in range(C):
                      y = ypool.tile([128, 512], F32, tag="y")
                      V.scalar_tensor_tensor(y, x[c], Sb[:, 0:1], tmp, OP.mult, OP.add)
                      o = opool.tile([128, 512], F32, tag="o")
                      eng = nc.vector if c == 0 else nc.gpsimd
                      eng.tensor_tensor(o, y, msk, OP.mult)

                      nc.gpsimd.indirect_dma_start(
                          out=out.rearrange("(n u) -> n u", u=1),
                          out_offset=bass.IndirectOffsetOnAxis(
                              ap=offt[:, c : c + 1], axis=0
                          ),
                          in_=o[:, :],
                          in_offset=None,
                      )

    nc.compile()
    return nc


_CACHE: dict = {}


def _get_compiled(n_imgs: int, repeat: int = 1):
    key = (n_imgs, repeat)
    if key not in _CACHE:
        _CACHE[key] = _build_kernel(n_imgs, repeat)
    return _CACHE[key]


def _pack_core_inputs(imgs, br, sat, con, tx, ty, cx, cy):
    """imgs: [n,3,256,256] f32 and per-image params for ONE core shard."""
    n = imgs.shape[0]
    prm = np.zeros((8, n), np.float32)
    prm[0] = br.reshape(n)
    prm[1] = sat.reshape(n)
    prm[2] = con.reshape(n)
    prm[3] = tx.reshape(n).astype(np.float32)
    prm[4] = ty.reshape(n).astype(np.float32)
    prm[5] = cx.reshape(n).astype(np.float32)
    prm[6] = cy.reshape(n).astype(np.float32)
    return {
        "imgs": np.ascontiguousarray(imgs.reshape(n * C, PLANE), dtype=np.float32),
        "prmr": np.ascontiguousarray(prm.reshape(1, 8 * n)),
        "prmc": np.ascontiguousarray(prm.T),
    }


def kernel(imgs, br, sat, con, tx, ty, cx, cy, _trace=False, _trace_kwargs=None, _repeat=1):
    imgs = np.asarray(imgs, dtype=np.float32)
    br = np.asarray(br, dtype=np.float32)
    sat = np.asarray(sat, dtype=np.float32)
    con = np.asarray(con, dtype=np.float32)
    tx = np.asarray(tx, dtype=np.int32)
    ty = np.asarray(ty, dtype=np.int32)
    cx = np.asarray(cx, dtype=np.int32)
    cy = np.asarray(cy, dtype=np.int32)

    n = IMGS_PER_CORE
    nc = _get_compiled(n, _repeat)

    in_maps = []
    for k in range(N_CORES):
        sl = slice(k * n, (k + 1) * n)
        in_maps.append(
            _pack_core_inputs(
                imgs[sl], br[sl], sat[sl], con[sl], tx[sl], ty[sl], cx[sl], cy[sl]
            )
        )

    res = run_bass_kernel_spmd(
        nc,
        in_maps,
        core_ids=list(range(N_CORES)),
        trace=_trace,
        **(_trace_kwargs or {}),
    )

    out = np.empty((B_FULL, C, H, W), np.float32)
    for k in range(N_CORES):
        flat = np.asarray(res.results[k]["out"]).reshape(-1)
        for j in range(n):
            for c in range(C):
                base = (j * C + c) * SLOT + MARG
                out[k * n + j, c] = flat[base : base + PLANE].reshape(H, W)
    if _trace:
        kernel._last_results = res
    return out


kernel._last_results = None



# revision 2
# speedup vs baseline: 115.6697x; 115.6697x over previous
"""Trainium2 Bass kernel v2 for nn_Augmenter (color jitter + translate + cutout).

Design vs v1:
  * Translation moved from scatter-STORE to gather-LOAD: host packs imgs
    (cast to bf16) into a flat buffer with 8224-element zero pads at head and
    tail; the whole translated window of one image is contiguous at flat
    offset s0 = 256*(tx-32) + (ty-32), so indirect loads with per-partition
    offsets (base + 512p) fetch the translated image directly. Out-of-range
    rows/cols load neighbor garbage, which the mask zeroes.
  * Mask built in OUTPUT coordinates: M = rv (x) cv - (rv&rc) (x) (cv&ccs)
    as two rank-1 PE matmuls per 256-col half (rv/cv = row/col validity of
    the translation, rc/ccs = cutout ranges). Exact {0,1} in f32 PSUM.
  * Per-image mean: DVE accum -> PE [128x128 all-ones] matmul fuses the
    partition reduce AND broadcast in one shot (no gpsimd partition ops).
  * Per-image params (A, Bp, ep, bpp, offb) broadcast to all partitions
    ONCE at setup with a single rank-1 PE matmul.
  * All image data on-chip in bf16 (rel-err budget 2e-2; measured ~6e-3).
    Output stored as bf16, host upcasts to f32.
  * Stores are dense static HWDGE DMAs to natural [n*3, 64K] layout: no
    margins, no reliance on pre-zeroed output buffers.

Math per image (same as reference):
  cf = con+0.5, A = cf*2*sat, Bp = (cf-A)/3, D = ep*SUM + bpp
  ep = (1-cf)/196608, bpp = br-0.5
  y = A*x + (Bp*MC + D);  out = y * M
"""

import numpy as np
import ml_dtypes

import concourse.bacc as bacc
import concourse.bass as bass
import concourse.mybir as mybir
import concourse.tile as tile

F32 = mybir.dt.float32
BF16 = mybir.dt.bfloat16
I32 = mybir.dt.int32
OP = mybir.AluOpType
AF = mybir.ActivationFunctionType

N_CORES = 8
B_FULL = 128
IMGS_PER_CORE = B_FULL // N_CORES  # 16
C, H, W = 3, 256, 256
PLANE = H * W  # 65536
PAD = 8224  # max |s0| = 256*32 + 32


def _build_kernel(n_imgs: int, repeat: int = 1, timing_unique_out: bool = False):
    nc = bacc.Bacc(
        "TRN2",
        target_bir_lowering=False,
        debug=False,
        enable_asserts=False,
        num_devices=N_CORES,
    )
    n_planes = n_imgs * C
    tot_in = 2 * PAD + n_planes * PLANE
    out_planes = n_planes * (repeat if timing_unique_out else 1)

    imgs_t = nc.dram_tensor("imgs", [tot_in], BF16, kind="ExternalInput")
    prmr_t = nc.dram_tensor("prmr", [1, 8 * n_imgs], F32, kind="ExternalInput")
    prmc_t = nc.dram_tensor("prmc", [n_imgs, 8], F32, kind="ExternalInput")
    out_t = nc.dram_tensor("out", [out_planes, PLANE], BF16, kind="ExternalOutput")
    imgs = imgs_t.ap()
    prmr = prmr_t.ap()
    prmc = prmc_t.ap()
    out = out_t.ap()

    n = n_imgs
    with tile.TileContext(nc) as tc:
        with (
            tc.tile_pool(name="const", bufs=1) as cpool,
            tc.tile_pool(name="xin", bufs=9) as xpool,
            tc.tile_pool(name="tsum", bufs=4) as tpool,
            tc.tile_pool(name="mc", bufs=4) as mcpool,
            tc.tile_pool(name="tmp", bufs=4) as tmppool,
            tc.tile_pool(name="msk", bufs=4) as mskpool,
            tc.tile_pool(name="yy", bufs=6) as ypool,
            tc.tile_pool(name="oo", bufs=6) as opool,
            tc.tile_pool(name="sm", bufs=8) as smpool,
            tc.tile_pool(name="vr", bufs=4) as vrpool,
            tc.tile_pool(name="ps", bufs=3, space="PSUM") as pspool,
            tc.tile_pool(name="psS", bufs=2, space="PSUM") as psspool,
        ):
            V = nc.vector

            # ---------------- one-time setup ----------------
            io_i = cpool.tile([n, 256], I32)
            nc.gpsimd.iota(io_i, pattern=[[1, 256]], base=0, channel_multiplier=0)
            IO = cpool.tile([n, 256], F32)
            V.tensor_copy(IO, io_i)

            ONES = cpool.tile([1, 128], F32)
            V.memset(ONES, 1.0)
            ONESQ = cpool.tile([128, 128], BF16)
            V.memset(ONESQ, 1.0)

            # iota over images (for per-image load offsets): [1, n] = 0..n-1
            ii_i = cpool.tile([1, n], I32)
            nc.gpsimd.iota(ii_i, pattern=[[1, n]], base=0, channel_multiplier=0)
            IIf = cpool.tile([1, n], F32)
            V.tensor_copy(IIf, ii_i)

            # row-layout params [1, 8n]: slot g*n + i
            Pr = cpool.tile([1, 8 * n], F32)
            nc.scalar.dma_start(Pr, prmr)
            BRr, SATr, CONr = (
                Pr[:, 0 * n : 1 * n],
                Pr[:, 1 * n : 2 * n],
                Pr[:, 2 * n : 3 * n],
            )
            TXr, TYr = Pr[:, 3 * n : 4 * n], Pr[:, 4 * n : 5 * n]

            # column-layout params [n, 8]
            Pc = cpool.tile([n, 8], F32)
            nc.scalar.dma_start(Pc, prmc)
            TXc, TYc = Pc[:, 3:4], Pc[:, 4:5]
            CXc, CYc = Pc[:, 5:6], Pc[:, 6:7]

            # --- P5 row [1, 5n]: per image i at 5i+: A, Bp, ep, bpp, offb ---
            P5 = cpool.tile([1, 5 * n], F32)
            A_r = P5[:, 0 : 5 * n : 5]
            Bp_r = P5[:, 1 : 5 * n : 5]
            ep_r = P5[:, 2 : 5 * n : 5]
            bpp_r = P5[:, 3 : 5 * n : 5]
            offb_r = P5[:, 4 : 5 * n : 5]
            ROW = cpool.tile([1, n], F32)
            cf = ROW[:, :]

            V.tensor_scalar(cf, CONr, 1.0, 0.5, OP.mult, OP.add)
            V.tensor_scalar(ep_r, cf, -1.0 / 196608.0, 1.0 / 196608.0, OP.mult, OP.add)
            V.tensor_scalar(bpp_r, BRr, 1.0, -0.5, OP.mult, OP.add)
            V.tensor_scalar(A_r, SATr, 2.0, None, OP.mult)
            V.tensor_tensor(A_r, cf, A_r, OP.mult)  # A = cf*2sat
            V.tensor_tensor(Bp_r, cf, A_r, OP.subtract)  # cf - A
            V.tensor_scalar(Bp_r, Bp_r, 1.0 / 3.0, None, OP.mult)
            V.tensor_scalar(offb_r, TXr, 256.0, None, OP.mult)
            V.tensor_tensor(offb_r, offb_r, TYr, OP.add)  # offb = 256tx+ty

            # full per-image dynamic load offsets: offi[i] = offb_i + 3i*PLANE
            OFFf = cpool.tile([1, n], F32)
            V.tensor_scalar(OFFf, IIf, float(3 * PLANE), None, OP.mult)
            V.tensor_tensor(OFFf, OFFf, offb_r, OP.add)
            OFFI = cpool.tile([1, n], I32)
            V.tensor_copy(OFFI, OFFf)

            # broadcast P5 to all partitions: PB [128, 5n]
            pb_ps = psspool.tile([128, 5 * n], F32, tag="pbps")
            nc.tensor.matmul(pb_ps, lhsT=ONES, rhs=P5, start=True, stop=True)
            PB = cpool.tile([128, 5 * n], F32)
            nc.scalar.activation(PB, pb_ps, AF.Copy)

            # --- mask vectors VEC [n, 1024]: rv | cv | -(rv&rc) | cv&ccs ---
            VEC = cpool.tile([n, 1024], BF16)
            rv_v = VEC[:, 0:256]
            cv_v = VEC[:, 256:512]
            nrc_v = VEC[:, 512:768]
            ccs_v = VEC[:, 768:1024]
            COL = cpool.tile([n, 4], F32)
            lo = COL[:, 0:1]
            hi = COL[:, 1:2]
            e1 = cpool.tile([n, 256], F32)

            # rv: 32-TX <= s <= 287-TX
            V.tensor_scalar(lo, TXc, -1.0, 32.0, OP.mult, OP.add)
            V.tensor_scalar(hi, TXc, -1.0, 287.0, OP.mult, OP.add)
            V.tensor_scalar(e1, IO, hi, None, OP.is_le)
            V.scalar_tensor_tensor(rv_v, IO, lo, e1, OP.is_ge, OP.logical_and)
            # cv: 32-TY <= s <= 287-TY
            V.tensor_scalar(lo, TYc, -1.0, 32.0, OP.mult, OP.add)
            V.tensor_scalar(hi, TYc, -1.0, 287.0, OP.mult, OP.add)
            V.tensor_scalar(e1, IO, hi, None, OP.is_le)
            V.scalar_tensor_tensor(cv_v, IO, lo, e1, OP.is_ge, OP.logical_and)
            # rc: max(0,CX-64) <= s <= min(255,CX+63); nrc = -(rc & rv)
            V.tensor_scalar(lo, CXc, 64.0, 0.0, OP.subtract, OP.max)
            V.tensor_scalar(hi, CXc, 63.0, 255.0, OP.add, OP.min)
            V.tensor_scalar(e1, IO, hi, None, OP.is_le)
            V.scalar_tensor_tensor(nrc_v, IO, lo, e1, OP.is_ge, OP.logical_and)
            V.tensor_tensor(nrc_v, nrc_v, rv_v, OP.mult)
            V.tensor_scalar(nrc_v, nrc_v, -1.0, None, OP.mult)
            # ccs: cut cols & cv
            V.tensor_scalar(lo, CYc, 64.0, 0.0, OP.subtract, OP.max)
            V.tensor_scalar(hi, CYc, 63.0, 255.0, OP.add, OP.min)
            V.tensor_scalar(e1, IO, hi, None, OP.is_le)
            V.scalar_tensor_tensor(ccs_v, IO, lo, e1, OP.is_ge, OP.logical_and)
            V.tensor_tensor(ccs_v, ccs_v, cv_v, OP.mult)

            # flatten images' vectors to rows at partitions 0/32/64 so PE can
            # read them (matmul base partition must be 0/32/64); 3 parallel
            # DMAs on different queues
            npp = (n + 2) // 3  # images per partition-row
            VECROW = cpool.tile([65, npp * 1024], BF16)
            for g, eng in zip(range(3), (nc.sync, nc.scalar, nc.gpsimd)):
                i0, i1 = g * npp, min((g + 1) * npp, n)
                if i0 < i1:
                    eng.dma_start(
                        VECROW[32 * g : 32 * g + 1, : (i1 - i0) * 1024],
                        VEC[i0:i1, :],
                    )

            def vec_row(i):
                g, j = i // npp, i % npp
                return VECROW[32 * g : 32 * g + 1, j * 1024 : (j + 1) * 1024]

            # ---------------- per-image pipeline ----------------
            for rep in range(repeat):
              for i in range(n):
                  # dynamic-base loads: window [offi, offi + 3*PLANE) is the
                  # whole translated image, contiguous in the padded buffer
                  regs = nc.alloc_registers(
                      f"roff_{rep}_{i}", engines=[mybir.EngineType.SP]
                  )
                  nc.regs_load(regs, OFFI[0:1, i : i + 1])
                  sv = nc.snap(
                      regs,
                      donate=True,
                      min_val=0,
                      max_val=3 * (n - 1) * PLANE + 16448,
                  )
                  win = imgs[bass.ds(sv, 3 * PLANE)]

                  x3 = xpool.tile([128, 3 * 512], BF16, tag="x", name=f"x{i}")
                  nc.sync.dma_start(
                      x3, win.rearrange("(c p f) -> p c f", c=3, p=128)
                  )
                  x = [x3[:, 512 * c : 512 * (c + 1)] for c in range(C)]

                  # channel sum + per-partition accum
                  t = tpool.tile([128, 512], BF16, tag="t")
                  nc.gpsimd.tensor_tensor(t, x[0], x[1], OP.add)
                  MC = mcpool.tile([128, 512], BF16, tag="mc")
                  mcp = smpool.tile([128, 1], BF16, tag="mcp")
                  V.scalar_tensor_tensor(
                      MC, t, 1.0, x[2], OP.mult, OP.add, accum_out=mcp
                  )

                  # SUM broadcast to all partitions: bsum = ONESQ^T @ mcp
                  bsum = psspool.tile([128, 1], F32, tag="bsum")
                  nc.tensor.matmul(bsum, lhsT=ONESQ, rhs=mcp, start=True, stop=True)
                  # D = ep*SUM + bpp on every partition
                  Dbc = smpool.tile([128, 1], F32, tag="dbc")
                  V.scalar_tensor_tensor(
                      Dbc,
                      bsum,
                      PB[:, 5 * i + 2 : 5 * i + 3],
                      PB[:, 5 * i + 3 : 5 * i + 4],
                      OP.mult,
                      OP.add,
                  )

                  # tmp = Bp*MC + D   (ScalarE)
                  tmp = tmppool.tile([128, 512], BF16, tag="tmp")
                  nc.scalar.activation(
                      tmp,
                      MC,
                      AF.Identity,
                      bias=Dbc[:, 0:1],
                      scale=PB[:, 5 * i + 1 : 5 * i + 2],
                  )

                  # mask: pm = rv (x) cv - (rv&rc) (x) (cv&ccs)
                  vr = vec_row(i)
                  pm = pspool.tile([128, 512], F32, tag="pm")
                  for b in range(2):
                      half = pm[:, b * 256 : (b + 1) * 256]
                      nc.tensor.matmul(
                          half,
                          lhsT=vr[:, b : 256 : 2],
                          rhs=vr[:, 256:512],
                          start=True,
                          stop=False,
                      )
                      nc.tensor.matmul(
                          half,
                          lhsT=vr[:, 512 + b : 768 : 2],
                          rhs=vr[:, 768:1024],
                          start=False,
                          stop=True,
                      )
                  msk = mskpool.tile([128, 512], BF16, tag="msk")
                  nc.scalar.activation(msk, pm, AF.Copy)

                  o3 = opool.tile([128, 3 * 512], BF16, tag="o")
                  y_engs = (nc.vector, nc.vector, nc.vector)
                  o_engs = (nc.vector, nc.gpsimd, nc.gpsimd)
                  for c in range(C):
                      y = ypool.tile([128, 512], BF16, tag="y")
                      y_engs[c].scalar_tensor_tensor(
                          y, x[c], PB[:, 5 * i : 5 * i + 1], tmp, OP.mult, OP.add
                      )
                      o_engs[c].tensor_tensor(
                          o3[:, 512 * c : 512 * (c + 1)], y, msk, OP.mult
                      )
                  ob = (rep * n_planes if timing_unique_out else 0) + 3 * i
                  nc.scalar.dma_start(
                      out[ob : ob + 3].rearrange("c (p f) -> p c f", p=128),
                      o3,
                  )

    nc.compile()
    return nc


_CACHE: dict = {}


def _get_compiled(n_imgs: int, repeat: int = 1):
    key = (n_imgs, repeat)
    if key not in _CACHE:
        _CACHE[key] = _build_kernel(n_imgs, repeat)
    return _CACHE[key]


def _pack_core_inputs(imgs, br, sat, con, tx, ty, cx, cy):
    """imgs: [n,3,256,256] f32 and per-image params for ONE core shard."""
    n = imgs.shape[0]
    buf = np.zeros(2 * PAD + n * C * PLANE, ml_dtypes.bfloat16)
    buf[PAD : PAD + n * C * PLANE] = imgs.reshape(-1).astype(ml_dtypes.bfloat16)
    prm = np.zeros((8, n), np.float32)
    prm[0] = br.reshape(n)
    prm[1] = sat.reshape(n)
    prm[2] = con.reshape(n)
    prm[3] = tx.reshape(n).astype(np.float32)
    prm[4] = ty.reshape(n).astype(np.float32)
    prm[5] = cx.reshape(n).astype(np.float32)
    prm[6] = cy.reshape(n).astype(np.float32)
    return {
        "imgs": buf,
        "prmr": np.ascontiguousarray(prm.reshape(1, 8 * n)),
        "prmc": np.ascontiguousarray(prm.T),
    }


def kernel(imgs, br, sat, con, tx, ty, cx, cy, _trace=False, _trace_kwargs=None, _repeat=1):
    from concourse.bass_utils import run_bass_kernel_spmd

    imgs = np.asarray(imgs, dtype=np.float32)
    br = np.asarray(br, dtype=np.float32)
    sat = np.asarray(sat, dtype=np.float32)
    con = np.asarray(con, dtype=np.float32)
    tx = np.asarray(tx, dtype=np.int32)
    ty = np.asarray(ty, dtype=np.int32)
    cx = np.asarray(cx, dtype=np.int32)
    cy = np.asarray(cy, dtype=np.int32)

    n = IMGS_PER_CORE
    nc = _get_compiled(n, _repeat)

    in_maps = []
    for k in range(N_CORES):
        sl = slice(k * n, (k + 1) * n)
        in_maps.append(
            _pack_core_inputs(
                imgs[sl], br[sl], sat[sl], con[sl], tx[sl], ty[sl], cx[sl], cy[sl]
            )
        )

    res = run_bass_kernel_spmd(
        nc,
        in_maps,
        core_ids=list(range(N_CORES)),
        trace=_trace,
        **(_trace_kwargs or {}),
    )

    out = np.empty((B_FULL, C, H, W), np.float32)
    for k in range(N_CORES):
        o = np.asarray(res.results[k]["out"]).reshape(n, C, H, W)
        out[k * n : (k + 1) * n] = o.astype(np.float32)
    if _trace:
        kernel._last_results = res
    return out


kernel._last_results = None


# revision 3
# speedup vs baseline: 123.6677x; 1.0691x over previous
"""Trainium2 Bass kernel v2 for nn_Augmenter (color jitter + translate + cutout).

Design vs v1:
  * Translation moved from scatter-STORE to gather-LOAD: host packs imgs
    (cast to bf16) into a flat buffer with 8224-element zero pads at head and
    tail; the whole translated window of one image is contiguous at flat
    offset s0 = 256*(tx-32) + (ty-32), so indirect loads with per-partition
    offsets (base + 512p) fetch the translated image directly. Out-of-range
    rows/cols load neighbor garbage, which the mask zeroes.
  * Mask built in OUTPUT coordinates: M = rv (x) cv - (rv&rc) (x) (cv&ccs)
    as two rank-1 PE matmuls per 256-col half (rv/cv = row/col validity of
    the translation, rc/ccs = cutout ranges). Exact {0,1} in f32 PSUM.
  * Per-image mean: DVE accum -> PE [128x128 all-ones] matmul fuses the
    partition reduce AND broadcast in one shot (no gpsimd partition ops).
  * Per-image params (A, Bp, ep, bpp, offb) broadcast to all partitions
    ONCE at setup with a single rank-1 PE matmul.
  * All image data on-chip in bf16 (rel-err budget 2e-2; measured ~6e-3).
    Output stored as bf16, host upcasts to f32.
  * Stores are dense static HWDGE DMAs to natural [n*3, 64K] layout: no
    margins, no reliance on pre-zeroed output buffers.

Math per image (same as reference):
  cf = con+0.5, A = cf*2*sat, Bp = (cf-A)/3, D = ep*SUM + bpp
  ep = (1-cf)/196608, bpp = br-0.5
  y = A*x + (Bp*MC + D);  out = y * M
"""

import numpy as np
import ml_dtypes

import concourse.bacc as bacc
import concourse.bass as bass
import concourse.mybir as mybir
import concourse.tile as tile

F32 = mybir.dt.float32
BF16 = mybir.dt.bfloat16
I32 = mybir.dt.int32
OP = mybir.AluOpType
AF = mybir.ActivationFunctionType

N_CORES = 8
B_FULL = 128
IMGS_PER_CORE = B_FULL // N_CORES  # 16
C, H, W = 3, 256, 256
PLANE = H * W  # 65536
PAD = 8224  # max |s0| = 256*32 + 32


def _build_kernel(n_imgs: int, repeat: int = 1, timing_unique_out: bool = False):
    nc = bacc.Bacc(
        "TRN2",
        target_bir_lowering=False,
        debug=False,
        enable_asserts=False,
        num_devices=N_CORES,
    )
    n_planes = n_imgs * C
    tot_in = 2 * PAD + n_planes * PLANE
    out_planes = n_planes * (repeat if timing_unique_out else 1)

    imgs_t = nc.dram_tensor("imgs", [tot_in], BF16, kind="ExternalInput")
    prmr_t = nc.dram_tensor("prmr", [1, 8 * n_imgs], F32, kind="ExternalInput")
    prmc_t = nc.dram_tensor("prmc", [n_imgs, 8], F32, kind="ExternalInput")
    out_t = nc.dram_tensor("out", [out_planes, PLANE], BF16, kind="ExternalOutput")
    imgs = imgs_t.ap()
    prmr = prmr_t.ap()
    prmc = prmc_t.ap()
    out = out_t.ap()

    n = n_imgs
    with tile.TileContext(nc) as tc:
        with (
            tc.tile_pool(name="const", bufs=1) as cpool,
            tc.tile_pool(name="xin", bufs=9) as xpool,
            tc.tile_pool(name="tsum", bufs=4) as tpool,
            tc.tile_pool(name="mc", bufs=4) as mcpool,
            tc.tile_pool(name="tmp", bufs=4) as tmppool,
            tc.tile_pool(name="msk", bufs=4) as mskpool,
            tc.tile_pool(name="yy", bufs=6) as ypool,
            tc.tile_pool(name="oo", bufs=6) as opool,
            tc.tile_pool(name="sm", bufs=8) as smpool,
            tc.tile_pool(name="vr", bufs=4) as vrpool,
            tc.tile_pool(name="ps", bufs=3, space="PSUM") as pspool,
            tc.tile_pool(name="psS", bufs=2, space="PSUM") as psspool,
        ):
            V = nc.vector

            # ---------------- one-time setup ----------------
            io_i = cpool.tile([n, 256], I32)
            nc.gpsimd.iota(io_i, pattern=[[1, 256]], base=0, channel_multiplier=0)
            IO = cpool.tile([n, 256], F32)
            V.tensor_copy(IO, io_i)

            ONES = cpool.tile([1, 128], F32)
            V.memset(ONES, 1.0)
            ONESQ = cpool.tile([128, 128], BF16)
            V.memset(ONESQ, 1.0)

            # iota over images (for per-image load offsets): [1, n] = 0..n-1
            ii_i = cpool.tile([1, n], I32)
            nc.gpsimd.iota(ii_i, pattern=[[1, n]], base=0, channel_multiplier=0)
            IIf = cpool.tile([1, n], F32)
            V.tensor_copy(IIf, ii_i)

            # row-layout params [1, 8n]: slot g*n + i
            Pr = cpool.tile([1, 8 * n], F32)
            nc.scalar.dma_start(Pr, prmr)
            BRr, SATr, CONr = (
                Pr[:, 0 * n : 1 * n],
                Pr[:, 1 * n : 2 * n],
                Pr[:, 2 * n : 3 * n],
            )
            TXr, TYr = Pr[:, 3 * n : 4 * n], Pr[:, 4 * n : 5 * n]

            # column-layout params [n, 8]
            Pc = cpool.tile([n, 8], F32)
            nc.scalar.dma_start(Pc, prmc)
            TXc, TYc = Pc[:, 3:4], Pc[:, 4:5]
            CXc, CYc = Pc[:, 5:6], Pc[:, 6:7]

            # --- P5 row [1, 5n]: per image i at 5i+: A, Bp, ep, bpp, offb ---
            P5 = cpool.tile([1, 5 * n], F32)
            A_r = P5[:, 0 : 5 * n : 5]
            Bp_r = P5[:, 1 : 5 * n : 5]
            ep_r = P5[:, 2 : 5 * n : 5]
            bpp_r = P5[:, 3 : 5 * n : 5]
            offb_r = P5[:, 4 : 5 * n : 5]
            ROW = cpool.tile([1, n], F32)
            cf = ROW[:, :]

            V.tensor_scalar(cf, CONr, 1.0, 0.5, OP.mult, OP.add)
            V.tensor_scalar(ep_r, cf, -1.0 / 196608.0, 1.0 / 196608.0, OP.mult, OP.add)
            V.tensor_scalar(bpp_r, BRr, 1.0, -0.5, OP.mult, OP.add)
            V.tensor_scalar(A_r, SATr, 2.0, None, OP.mult)
            V.tensor_tensor(A_r, cf, A_r, OP.mult)  # A = cf*2sat
            V.tensor_tensor(Bp_r, cf, A_r, OP.subtract)  # cf - A
            V.tensor_scalar(Bp_r, Bp_r, 1.0 / 3.0, None, OP.mult)
            V.tensor_scalar(offb_r, TXr, 256.0, None, OP.mult)
            V.tensor_tensor(offb_r, offb_r, TYr, OP.add)  # offb = 256tx+ty

            # full per-image dynamic load offsets: offi[i] = offb_i + 3i*PLANE
            OFFf = cpool.tile([1, n], F32)
            V.tensor_scalar(OFFf, IIf, float(3 * PLANE), None, OP.mult)
            V.tensor_tensor(OFFf, OFFf, offb_r, OP.add)
            OFFI = cpool.tile([1, n], I32)
            V.tensor_copy(OFFI, OFFf)

            # broadcast P5 to all partitions: PB [128, 5n]
            pb_ps = psspool.tile([128, 5 * n], F32, tag="pbps")
            nc.tensor.matmul(pb_ps, lhsT=ONES, rhs=P5, start=True, stop=True)
            PB = cpool.tile([128, 5 * n], F32)
            nc.scalar.activation(PB, pb_ps, AF.Copy)

            # --- mask vectors VEC [n, 1024]: rv | cv | -(rv&rc) | cv&ccs ---
            VEC = cpool.tile([n, 1024], BF16)
            rv_v = VEC[:, 0:256]
            cv_v = VEC[:, 256:512]
            nrc_v = VEC[:, 512:768]
            ccs_v = VEC[:, 768:1024]
            COL = cpool.tile([n, 4], F32)
            lo = COL[:, 0:1]
            hi = COL[:, 1:2]
            e1 = cpool.tile([n, 256], F32)

            # rv: 32-TX <= s <= 287-TX
            V.tensor_scalar(lo, TXc, -1.0, 32.0, OP.mult, OP.add)
            V.tensor_scalar(hi, TXc, -1.0, 287.0, OP.mult, OP.add)
            V.tensor_scalar(e1, IO, hi, None, OP.is_le)
            V.scalar_tensor_tensor(rv_v, IO, lo, e1, OP.is_ge, OP.logical_and)
            # cv: 32-TY <= s <= 287-TY
            V.tensor_scalar(lo, TYc, -1.0, 32.0, OP.mult, OP.add)
            V.tensor_scalar(hi, TYc, -1.0, 287.0, OP.mult, OP.add)
            V.tensor_scalar(e1, IO, hi, None, OP.is_le)
            V.scalar_tensor_tensor(cv_v, IO, lo, e1, OP.is_ge, OP.logical_and)
            # rc: max(0,CX-64) <= s <= min(255,CX+63); nrc = -(rc & rv)
            V.tensor_scalar(lo, CXc, 64.0, 0.0, OP.subtract, OP.max)
            V.tensor_scalar(hi, CXc, 63.0, 255.0, OP.add, OP.min)
            V.tensor_scalar(e1, IO, hi, None, OP.is_le)
            V.scalar_tensor_tensor(nrc_v, IO, lo, e1, OP.is_ge, OP.logical_and)
            V.tensor_tensor(nrc_v, nrc_v, rv_v, OP.mult)
            V.tensor_scalar(nrc_v, nrc_v, -1.0, None, OP.mult)
            # ccs: cut cols & cv
            V.tensor_scalar(lo, CYc, 64.0, 0.0, OP.subtract, OP.max)
            V.tensor_scalar(hi, CYc, 63.0, 255.0, OP.add, OP.min)
            V.tensor_scalar(e1, IO, hi, None, OP.is_le)
            V.scalar_tensor_tensor(ccs_v, IO, lo, e1, OP.is_ge, OP.logical_and)
            V.tensor_tensor(ccs_v, ccs_v, cv_v, OP.mult)

            # flatten images' vectors to rows at partitions 0/32/64 so PE can
            # read them (matmul base partition must be 0/32/64); 3 parallel
            # DMAs on different queues
            npp = (n + 2) // 3  # images per partition-row
            VECROW = cpool.tile([65, npp * 1024], BF16)
            for g, eng in zip(range(3), (nc.sync, nc.scalar, nc.gpsimd)):
                i0, i1 = g * npp, min((g + 1) * npp, n)
                if i0 < i1:
                    eng.dma_start(
                        VECROW[32 * g : 32 * g + 1, : (i1 - i0) * 1024],
                        VEC[i0:i1, :],
                    )

            def vec_row(i):
                g, j = i // npp, i % npp
                return VECROW[32 * g : 32 * g + 1, j * 1024 : (j + 1) * 1024]

            # ---------------- per-image pipeline ----------------
            for rep in range(repeat):
              for i in range(n):
                  # dynamic-base loads: window [offi, offi + 3*PLANE) is the
                  # whole translated image, contiguous in the padded buffer
                  regs = nc.alloc_registers(
                      f"roff_{rep}_{i}",
                      engines=[
                          mybir.EngineType.SP
                          if i % 2 == 0
                          else mybir.EngineType.Activation
                      ],
                  )
                  nc.regs_load(regs, OFFI[0:1, i : i + 1])
                  sv = nc.snap(
                      regs,
                      donate=True,
                      min_val=0,
                      max_val=3 * (n - 1) * PLANE + 16448,
                  )
                  win = imgs[bass.ds(sv, 3 * PLANE)]

                  x3 = xpool.tile([128, 3 * 512], BF16, tag="x", name=f"x{i}")
                  ld_eng = nc.sync if i % 2 == 0 else nc.scalar
                  ld_eng.dma_start(
                      x3, win.rearrange("(c p f) -> p c f", c=3, p=128)
                  )
                  x = [x3[:, 512 * c : 512 * (c + 1)] for c in range(C)]

                  # channel sum + per-partition accum
                  t = tpool.tile([128, 512], BF16, tag="t")
                  nc.gpsimd.tensor_tensor(t, x[0], x[1], OP.add)
                  MC = mcpool.tile([128, 512], BF16, tag="mc")
                  mcp = smpool.tile([128, 1], BF16, tag="mcp")
                  V.scalar_tensor_tensor(
                      MC, t, 1.0, x[2], OP.mult, OP.add, accum_out=mcp
                  )

                  # SUM broadcast to all partitions: bsum = ONESQ^T @ mcp
                  bsum = psspool.tile([128, 1], F32, tag="bsum")
                  nc.tensor.matmul(bsum, lhsT=ONESQ, rhs=mcp, start=True, stop=True)
                  # D = ep*SUM + bpp on every partition
                  Dbc = smpool.tile([128, 1], F32, tag="dbc")
                  V.scalar_tensor_tensor(
                      Dbc,
                      bsum,
                      PB[:, 5 * i + 2 : 5 * i + 3],
                      PB[:, 5 * i + 3 : 5 * i + 4],
                      OP.mult,
                      OP.add,
                  )

                  # tmp = Bp*MC + D   (ScalarE)
                  tmp = tmppool.tile([128, 512], BF16, tag="tmp")
                  nc.scalar.activation(
                      tmp,
                      MC,
                      AF.Identity,
                      bias=Dbc[:, 0:1],
                      scale=PB[:, 5 * i + 1 : 5 * i + 2],
                  )

                  # mask: pm = rv (x) cv - (rv&rc) (x) (cv&ccs)
                  vr = vec_row(i)
                  pm = pspool.tile([128, 512], F32, tag="pm")
                  for b in range(2):
                      half = pm[:, b * 256 : (b + 1) * 256]
                      nc.tensor.matmul(
                          half,
                          lhsT=vr[:, b : 256 : 2],
                          rhs=vr[:, 256:512],
                          start=True,
                          stop=False,
                      )
                      nc.tensor.matmul(
                          half,
                          lhsT=vr[:, 512 + b : 768 : 2],
                          rhs=vr[:, 768:1024],
                          start=False,
                          stop=True,
                      )
                  msk = mskpool.tile([128, 512], BF16, tag="msk")
                  nc.scalar.activation(msk, pm, AF.Copy)

                  o3 = opool.tile([128, 3 * 512], BF16, tag="o")
                  y_engs = (nc.vector, nc.vector, nc.vector)
                  o_engs = (nc.vector, nc.gpsimd, nc.gpsimd)
                  for c in range(C):
                      y = ypool.tile([128, 512], BF16, tag="y")
                      y_engs[c].scalar_tensor_tensor(
                          y, x[c], PB[:, 5 * i : 5 * i + 1], tmp, OP.mult, OP.add
                      )
                      o_engs[c].tensor_tensor(
                          o3[:, 512 * c : 512 * (c + 1)], y, msk, OP.mult
                      )
                  ob = (rep * n_planes if timing_unique_out else 0) + 3 * i
                  nc.scalar.dma_start(
                      out[ob : ob + 3].rearrange("c (p f) -> p c f", p=128),
                      o3,
                  )

    nc.compile()
    return nc


_CACHE: dict = {}


def _get_compiled(n_imgs: int, repeat: int = 1):
    key = (n_imgs, repeat)
    if key not in _CACHE:
        _CACHE[key] = _build_kernel(n_imgs, repeat)
    return _CACHE[key]


def _pack_core_inputs(imgs, br, sat, con, tx, ty, cx, cy):
    """imgs: [n,3,256,256] f32 and per-image params for ONE core shard."""
    n = imgs.shape[0]
    buf = np.zeros(2 * PAD + n * C * PLANE, ml_dtypes.bfloat16)
    buf[PAD : PAD + n * C * PLANE] = imgs.reshape(-1).astype(ml_dtypes.bfloat16)
    prm = np.zeros((8, n), np.float32)
    prm[0] = br.reshape(n)
    prm[1] = sat.reshape(n)
    prm[2] = con.reshape(n)
    prm[3] = tx.reshape(n).astype(np.float32)
    prm[4] = ty.reshape(n).astype(np.float32)
    prm[5] = cx.reshape(n).astype(np.float32)
    prm[6] = cy.reshape(n).astype(np.float32)
    return {
        "imgs": buf,
        "prmr": np.ascontiguousarray(prm.reshape(1, 8 * n)),
        "prmc": np.ascontiguousarray(prm.T),
    }


def kernel(imgs, br, sat, con, tx, ty, cx, cy, _trace=False, _trace_kwargs=None, _repeat=1):
    from concourse.bass_utils import run_bass_kernel_spmd

    imgs = np.asarray(imgs, dtype=np.float32)
    br = np.asarray(br, dtype=np.float32)
    sat = np.asarray(sat, dtype=np.float32)
    con = np.asarray(con, dtype=np.float32)
    tx = np.asarray(tx, dtype=np.int32)
    ty = np.asarray(ty, dtype=np.int32)
    cx = np.asarray(cx, dtype=np.int32)
    cy = np.asarray(cy, dtype=np.int32)

    n = IMGS_PER_CORE
    nc = _get_compiled(n, _repeat)

    in_maps = []
    for k in range(N_CORES):
        sl = slice(k * n, (k + 1) * n)
        in_maps.append(
            _pack_core_inputs(
                imgs[sl], br[sl], sat[sl], con[sl], tx[sl], ty[sl], cx[sl], cy[sl]
            )
        )

    res = run_bass_kernel_spmd(
        nc,
        in_maps,
        core_ids=list(range(N_CORES)),
        trace=_trace,
        **(_trace_kwargs or {}),
    )

    out = np.empty((B_FULL, C, H, W), np.float32)
    for k in range(N_CORES):
        o = np.asarray(res.results[k]["out"]).reshape(n, C, H, W)
        out[k * n : (k + 1) * n] = o.astype(np.float32)
    if _trace:
        kernel._last_results = res
    return out


kernel._last_results = None


# revision 4
# speedup vs baseline: 126.4269x; 1.0223x over previous
"""Trainium2 Bass kernel v2 for nn_Augmenter (color jitter + translate + cutout).

Design vs v1:
  * Translation moved from scatter-STORE to gather-LOAD: host packs imgs
    (cast to bf16) into a flat buffer with 8224-element zero pads at head and
    tail; the whole translated window of one image is contiguous at flat
    offset s0 = 256*(tx-32) + (ty-32), so indirect loads with per-partition
    offsets (base + 512p) fetch the translated image directly. Out-of-range
    rows/cols load neighbor garbage, which the mask zeroes.
  * Mask built in OUTPUT coordinates: M = rv (x) cv - (rv&rc) (x) (cv&ccs)
    as two rank-1 PE matmuls per 256-col half (rv/cv = row/col validity of
    the translation, rc/ccs = cutout ranges). Exact {0,1} in f32 PSUM.
  * Per-image mean: DVE accum -> PE [128x128 all-ones] matmul fuses the
    partition reduce AND broadcast in one shot (no gpsimd partition ops).
  * Per-image params (A, Bp, ep, bpp, offb) broadcast to all partitions
    ONCE at setup with a single rank-1 PE matmul.
  * All image data on-chip in bf16 (rel-err budget 2e-2; measured ~6e-3).
    Output stored as bf16, host upcasts to f32.
  * Stores are dense static HWDGE DMAs to natural [n*3, 64K] layout: no
    margins, no reliance on pre-zeroed output buffers.

Math per image (same as reference):
  cf = con+0.5, A = cf*2*sat, Bp = (cf-A)/3, D = ep*SUM + bpp
  ep = (1-cf)/196608, bpp = br-0.5
  y = A*x + (Bp*MC + D);  out = y * M
"""

import numpy as np
import ml_dtypes

import concourse.bacc as bacc
import concourse.bass as bass
import concourse.mybir as mybir
import concourse.tile as tile

F32 = mybir.dt.float32
BF16 = mybir.dt.bfloat16
I32 = mybir.dt.int32
OP = mybir.AluOpType
AF = mybir.ActivationFunctionType

N_CORES = 8
B_FULL = 128
IMGS_PER_CORE = B_FULL // N_CORES  # 16
C, H, W = 3, 256, 256
PLANE = H * W  # 65536
PAD = 8224  # max |s0| = 256*32 + 32


def _build_kernel(n_imgs: int, repeat: int = 1, timing_unique_out: bool = False):
    nc = bacc.Bacc(
        "TRN2",
        target_bir_lowering=False,
        debug=False,
        enable_asserts=False,
        num_devices=N_CORES,
    )
    n_planes = n_imgs * C
    tot_in = 2 * PAD + n_planes * PLANE
    out_planes = n_planes * (repeat if timing_unique_out else 1)

    imgs_t = nc.dram_tensor("imgs", [tot_in], BF16, kind="ExternalInput")
    prmr_t = nc.dram_tensor("prmr", [1, 8 * n_imgs], F32, kind="ExternalInput")
    prmc_t = nc.dram_tensor("prmc", [n_imgs, 8], F32, kind="ExternalInput")
    out_t = nc.dram_tensor("out", [out_planes, PLANE], BF16, kind="ExternalOutput")
    imgs = imgs_t.ap()
    prmr = prmr_t.ap()
    prmc = prmc_t.ap()
    out = out_t.ap()

    n = n_imgs
    with tile.TileContext(nc) as tc:
        with (
            tc.tile_pool(name="const", bufs=1) as cpool,
            tc.tile_pool(name="xin", bufs=9) as xpool,
            tc.tile_pool(name="tsum", bufs=4) as tpool,
            tc.tile_pool(name="mc", bufs=4) as mcpool,
            tc.tile_pool(name="tmp", bufs=4) as tmppool,
            tc.tile_pool(name="msk", bufs=4) as mskpool,
            tc.tile_pool(name="yy", bufs=6) as ypool,
            tc.tile_pool(name="oo", bufs=6) as opool,
            tc.tile_pool(name="sm", bufs=8) as smpool,
            tc.tile_pool(name="vr", bufs=4) as vrpool,
            tc.tile_pool(name="ps", bufs=3, space="PSUM") as pspool,
            tc.tile_pool(name="psS", bufs=2, space="PSUM") as psspool,
        ):
            V = nc.vector

            # ---------------- one-time setup ----------------
            io_i = cpool.tile([n, 256], I32)
            nc.gpsimd.iota(io_i, pattern=[[1, 256]], base=0, channel_multiplier=0)
            IO = cpool.tile([n, 256], F32)
            V.tensor_copy(IO, io_i)

            ONES = cpool.tile([1, 128], F32)
            V.memset(ONES, 1.0)
            ONESQ = cpool.tile([128, 128], BF16)
            V.memset(ONESQ, 1.0)

            # iota over images (for per-image load offsets): [1, n] = 0..n-1
            ii_i = cpool.tile([1, n], I32)
            nc.gpsimd.iota(ii_i, pattern=[[1, n]], base=0, channel_multiplier=0)
            IIf = cpool.tile([1, n], F32)
            V.tensor_copy(IIf, ii_i)

            # row-layout params [1, 8n]: slot g*n + i
            Pr = cpool.tile([1, 8 * n], F32)
            nc.sync.dma_start(Pr, prmr)
            BRr, SATr, CONr = (
                Pr[:, 0 * n : 1 * n],
                Pr[:, 1 * n : 2 * n],
                Pr[:, 2 * n : 3 * n],
            )
            TXr, TYr = Pr[:, 3 * n : 4 * n], Pr[:, 4 * n : 5 * n]

            # column-layout params [n, 8]
            Pc = cpool.tile([n, 8], F32)
            nc.sync.dma_start(Pc, prmc)
            TXc, TYc = Pc[:, 3:4], Pc[:, 4:5]
            CXc, CYc = Pc[:, 5:6], Pc[:, 6:7]

            # --- P5 row [1, 5n]: per image i at 5i+: A, Bp, ep, bpp, offb ---
            P5 = cpool.tile([1, 5 * n], F32)
            A_r = P5[:, 0 : 5 * n : 5]
            Bp_r = P5[:, 1 : 5 * n : 5]
            ep_r = P5[:, 2 : 5 * n : 5]
            bpp_r = P5[:, 3 : 5 * n : 5]
            offb_r = P5[:, 4 : 5 * n : 5]
            ROW = cpool.tile([1, n], F32)
            cf = ROW[:, :]

            V.tensor_scalar(cf, CONr, 1.0, 0.5, OP.mult, OP.add)
            V.tensor_scalar(ep_r, cf, -1.0 / 196608.0, 1.0 / 196608.0, OP.mult, OP.add)
            V.tensor_scalar(bpp_r, BRr, 1.0, -0.5, OP.mult, OP.add)
            V.tensor_scalar(A_r, SATr, 2.0, None, OP.mult)
            V.tensor_tensor(A_r, cf, A_r, OP.mult)  # A = cf*2sat
            V.tensor_tensor(Bp_r, cf, A_r, OP.subtract)  # cf - A
            V.tensor_scalar(Bp_r, Bp_r, 1.0 / 3.0, None, OP.mult)
            V.tensor_scalar(offb_r, TXr, 256.0, None, OP.mult)
            V.tensor_tensor(offb_r, offb_r, TYr, OP.add)  # offb = 256tx+ty

            # full per-image dynamic load offsets: offi[i] = offb_i + 3i*PLANE
            OFFf = cpool.tile([1, n], F32)
            V.tensor_scalar(OFFf, IIf, float(3 * PLANE), None, OP.mult)
            V.tensor_tensor(OFFf, OFFf, offb_r, OP.add)
            OFFI = cpool.tile([1, n], I32)
            V.tensor_copy(OFFI, OFFf)

            # broadcast P5 to all partitions: PB [128, 5n]
            pb_ps = psspool.tile([128, 5 * n], F32, tag="pbps")
            nc.tensor.matmul(pb_ps, lhsT=ONES, rhs=P5, start=True, stop=True)
            PB = cpool.tile([128, 5 * n], F32)
            nc.scalar.activation(PB, pb_ps, AF.Copy)

            # --- mask vectors VEC [n, 1024]: rv | cv | -(rv&rc) | cv&ccs ---
            VEC = cpool.tile([n, 1024], BF16)
            rv_v = VEC[:, 0:256]
            cv_v = VEC[:, 256:512]
            nrc_v = VEC[:, 512:768]
            ccs_v = VEC[:, 768:1024]
            COL = cpool.tile([n, 4], F32)
            lo = COL[:, 0:1]
            hi = COL[:, 1:2]
            e1 = cpool.tile([n, 256], F32)

            # rv: 32-TX <= s <= 287-TX
            V.tensor_scalar(lo, TXc, -1.0, 32.0, OP.mult, OP.add)
            V.tensor_scalar(hi, TXc, -1.0, 287.0, OP.mult, OP.add)
            V.tensor_scalar(e1, IO, hi, None, OP.is_le)
            V.scalar_tensor_tensor(rv_v, IO, lo, e1, OP.is_ge, OP.logical_and)
            # cv: 32-TY <= s <= 287-TY
            V.tensor_scalar(lo, TYc, -1.0, 32.0, OP.mult, OP.add)
            V.tensor_scalar(hi, TYc, -1.0, 287.0, OP.mult, OP.add)
            V.tensor_scalar(e1, IO, hi, None, OP.is_le)
            V.scalar_tensor_tensor(cv_v, IO, lo, e1, OP.is_ge, OP.logical_and)
            # rc: max(0,CX-64) <= s <= min(255,CX+63); nrc = -(rc & rv)
            V.tensor_scalar(lo, CXc, 64.0, 0.0, OP.subtract, OP.max)
            V.tensor_scalar(hi, CXc, 63.0, 255.0, OP.add, OP.min)
            V.tensor_scalar(e1, IO, hi, None, OP.is_le)
            V.scalar_tensor_tensor(nrc_v, IO, lo, e1, OP.is_ge, OP.logical_and)
            V.tensor_tensor(nrc_v, nrc_v, rv_v, OP.mult)
            V.tensor_scalar(nrc_v, nrc_v, -1.0, None, OP.mult)
            # ccs: cut cols & cv
            V.tensor_scalar(lo, CYc, 64.0, 0.0, OP.subtract, OP.max)
            V.tensor_scalar(hi, CYc, 63.0, 255.0, OP.add, OP.min)
            V.tensor_scalar(e1, IO, hi, None, OP.is_le)
            V.scalar_tensor_tensor(ccs_v, IO, lo, e1, OP.is_ge, OP.logical_and)
            V.tensor_tensor(ccs_v, ccs_v, cv_v, OP.mult)

            # flatten images' vectors to rows at partitions 0/32/64 so PE can
            # read them (matmul base partition must be 0/32/64); 3 parallel
            # DMAs on different queues
            npp = (n + 2) // 3  # images per partition-row
            VECROW = cpool.tile([65, npp * 1024], BF16)
            for g, eng in zip(range(3), (nc.sync, nc.scalar, nc.gpsimd)):
                i0, i1 = g * npp, min((g + 1) * npp, n)
                if i0 < i1:
                    eng.dma_start(
                        VECROW[32 * g : 32 * g + 1, : (i1 - i0) * 1024],
                        VEC[i0:i1, :],
                    )

            def vec_row(i):
                g, j = i // npp, i % npp
                return VECROW[32 * g : 32 * g + 1, j * 1024 : (j + 1) * 1024]

            # ---------------- per-image pipeline ----------------
            for rep in range(repeat):
              for i in range(n):
                  # dynamic-base loads: window [offi, offi + 3*PLANE) is the
                  # whole translated image, contiguous in the padded buffer
                  regs = nc.alloc_registers(
                      f"roff_{rep}_{i}",
                      engines=[
                          mybir.EngineType.SP
                          if i % 2 == 0
                          else mybir.EngineType.Activation
                      ],
                  )
                  nc.regs_load(regs, OFFI[0:1, i : i + 1])
                  sv = nc.snap(
                      regs,
                      donate=True,
                      min_val=0,
                      max_val=3 * (n - 1) * PLANE + 16448,
                  )
                  win = imgs[bass.ds(sv, 3 * PLANE)]

                  x3 = xpool.tile([128, 3 * 512], BF16, tag="x", name=f"x{i}")
                  ld_eng = nc.sync if i % 2 == 0 else nc.scalar
                  ld_eng.dma_start(
                      x3, win.rearrange("(c p f) -> p c f", c=3, p=128)
                  )
                  x = [x3[:, 512 * c : 512 * (c + 1)] for c in range(C)]

                  # channel sum + per-partition accum
                  t = tpool.tile([128, 512], BF16, tag="t")
                  nc.gpsimd.tensor_tensor(t, x[0], x[1], OP.add)
                  MC = mcpool.tile([128, 512], BF16, tag="mc")
                  mcp = smpool.tile([128, 1], BF16, tag="mcp")
                  V.scalar_tensor_tensor(
                      MC, t, 1.0, x[2], OP.mult, OP.add, accum_out=mcp
                  )

                  # SUM broadcast to all partitions: bsum = ONESQ^T @ mcp
                  bsum = psspool.tile([128, 1], F32, tag="bsum")
                  nc.tensor.matmul(bsum, lhsT=ONESQ, rhs=mcp, start=True, stop=True)
                  # D = ep*SUM + bpp on every partition
                  Dbc = smpool.tile([128, 1], F32, tag="dbc")
                  V.scalar_tensor_tensor(
                      Dbc,
                      bsum,
                      PB[:, 5 * i + 2 : 5 * i + 3],
                      PB[:, 5 * i + 3 : 5 * i + 4],
                      OP.mult,
                      OP.add,
                  )

                  # tmp = Bp*MC + D   (ScalarE)
                  tmp = tmppool.tile([128, 512], BF16, tag="tmp")
                  nc.scalar.activation(
                      tmp,
                      MC,
                      AF.Identity,
                      bias=Dbc[:, 0:1],
                      scale=PB[:, 5 * i + 1 : 5 * i + 2],
                  )

                  # mask: pm = rv (x) cv - (rv&rc) (x) (cv&ccs)
                  vr = vec_row(i)
                  pm = pspool.tile([128, 512], F32, tag="pm")
                  for b in range(2):
                      half = pm[:, b * 256 : (b + 1) * 256]
                      nc.tensor.matmul(
                          half,
                          lhsT=vr[:, b : 256 : 2],
                          rhs=vr[:, 256:512],
                          start=True,
                          stop=False,
                      )
                      nc.tensor.matmul(
                          half,
                          lhsT=vr[:, 512 + b : 768 : 2],
                          rhs=vr[:, 768:1024],
                          start=False,
                          stop=True,
                      )
                  msk = mskpool.tile([128, 512], BF16, tag="msk")
                  nc.scalar.activation(msk, pm, AF.Copy)

                  o3 = opool.tile([128, 3 * 512], BF16, tag="o")
                  y_engs = (nc.vector, nc.vector, nc.vector)
                  o_engs = (nc.vector, nc.gpsimd, nc.gpsimd)
                  for c in range(C):
                      y = ypool.tile([128, 512], BF16, tag="y")
                      y_engs[c].scalar_tensor_tensor(
                          y, x[c], PB[:, 5 * i : 5 * i + 1], tmp, OP.mult, OP.add
                      )
                      o_engs[c].tensor_tensor(
                          o3[:, 512 * c : 512 * (c + 1)], y, msk, OP.mult
                      )
                  ob = (rep * n_planes if timing_unique_out else 0) + 3 * i
                  nc.scalar.dma_start(
                      out[ob : ob + 3].rearrange("c (p f) -> p c f", p=128),
                      o3,
                  )

    nc.compile()
    return nc


_CACHE: dict = {}


def _get_compiled(n_imgs: int, repeat: int = 1):
    key = (n_imgs, repeat)
    if key not in _CACHE:
        _CACHE[key] = _build_kernel(n_imgs, repeat)
    return _CACHE[key]


def _pack_core_inputs(imgs, br, sat, con, tx, ty, cx, cy):
    """imgs: [n,3,256,256] f32 and per-image params for ONE core shard."""
    n = imgs.shape[0]
    buf = np.zeros(2 * PAD + n * C * PLANE, ml_dtypes.bfloat16)
    buf[PAD : PAD + n * C * PLANE] = imgs.reshape(-1).astype(ml_dtypes.bfloat16)
    prm = np.zeros((8, n), np.float32)
    prm[0] = br.reshape(n)
    prm[1] = sat.reshape(n)
    prm[2] = con.reshape(n)
    prm[3] = tx.reshape(n).astype(np.float32)
    prm[4] = ty.reshape(n).astype(np.float32)
    prm[5] = cx.reshape(n).astype(np.float32)
    prm[6] = cy.reshape(n).astype(np.float32)
    return {
        "imgs": buf,
        "prmr": np.ascontiguousarray(prm.reshape(1, 8 * n)),
        "prmc": np.ascontiguousarray(prm.T),
    }


def kernel(imgs, br, sat, con, tx, ty, cx, cy, _trace=False, _trace_kwargs=None, _repeat=1):
    from concourse.bass_utils import run_bass_kernel_spmd

    imgs = np.asarray(imgs, dtype=np.float32)
    br = np.asarray(br, dtype=np.float32)
    sat = np.asarray(sat, dtype=np.float32)
    con = np.asarray(con, dtype=np.float32)
    tx = np.asarray(tx, dtype=np.int32)
    ty = np.asarray(ty, dtype=np.int32)
    cx = np.asarray(cx, dtype=np.int32)
    cy = np.asarray(cy, dtype=np.int32)

    n = IMGS_PER_CORE
    nc = _get_compiled(n, _repeat)

    in_maps = []
    for k in range(N_CORES):
        sl = slice(k * n, (k + 1) * n)
        in_maps.append(
            _pack_core_inputs(
                imgs[sl], br[sl], sat[sl], con[sl], tx[sl], ty[sl], cx[sl], cy[sl]
            )
        )

    res = run_bass_kernel_spmd(
        nc,
        in_maps,
        core_ids=list(range(N_CORES)),
        trace=_trace,
        **(_trace_kwargs or {}),
    )

    out = np.empty((B_FULL, C, H, W), np.float32)
    for k in range(N_CORES):
        o = np.asarray(res.results[k]["out"]).reshape(n, C, H, W)
        out[k * n : (k + 1) * n] = o.astype(np.float32)
    if _trace:
        kernel._last_results = res
    return out


kernel._last_results = None
